# revision 13
# baseline (speedup 1.0000x reference)
"""MoE++ layer (nn_MoEPlusPlusLayer) on 8 Trainium2 NeuronCores.

Strategy (expert-parallel, per the sharding hint):
  - Host computes the fp32 routing math once to DISPATCH tokens by expert id
    (the sharding step): each of the 8 cores owns one expert's FFN weights and
    receives that expert's assigned tokens as a capacity-padded batch.
  - Gates/biases are folded exactly into the token batch: xg = [g * x^T; g; 0]
    and W1a = [W1; b1; 0], W2a = [W2; b2; 0], using relu(g*z) = g*relu(z) for
    g >= 0. The device FFN is then pure matmul -> relu -> matmul (float32r on
    the PE array at full rate, ~1e-4 relative error).
  - Each core also computes the routing OUTPUTS (router_logits, conf,
    selected_weights, selected_indices) for its 1/8 slice of tokens in fp32
    (data-parallel over the batch, small nets replicated).
  - Host unshards: concatenates routing outputs and combines the two gated
    expert contributions per token with two gathers + add.
"""

import math

import numpy as np

import concourse.bass as bass
import concourse.mybir as mybir
import concourse.tile as tile
from concourse import bacc
from concourse.bass_utils import run_bass_kernel_spmd

# Model dims (fixed by the problem)
B, S, H = 4, 2048, 1024
E, I = 8, 4096
TOP_K = 2
MIN_K, MAX_K = 1, 4
TEMPERATURE = 1.0

N_CORES = 8
N_TOK = B * S                  # 8192
TPC = N_TOK // N_CORES         # tokens per core for the routing math
HC = H // 2                    # confidence hidden dim (512)
KT_H = H // 128                # 8 k-tiles over H
KT_HA = KT_H + 1               # 9 k-tiles over augmented H (bias row)
KT_I = I // 128                # 32 k-tiles over I
KT_IA = KT_I + 1               # 33 k-tiles over augmented I (bias row)
KT_C = HC // 128               # 4 k-tiles over confidence hidden
HA = KT_HA * 128               # 1152 augmented input rows
IA = KT_IA * 128               # 4224 augmented intermediate rows
CHUNK = 512                    # token chunk (= matmul moving dim = psum bank)

F32 = mybir.dt.float32
F32R = mybir.dt.float32r
U32 = mybir.dt.uint32
I32 = mybir.dt.int32
AF = mybir.ActivationFunctionType
AX = mybir.AxisListType
ALU = mybir.AluOpType

_prog_cache: dict = {}


def _emit_routing(nc, tc, io):
    """Routing outputs for this core's TPC tokens. All per-token-tile math is
    batched into 3D [128, n_tt, E] tiles to minimize instruction count and
    ACT function-table switches."""
    xr, wc1, bc1v, wc2, bc2b, wr, brb = (
        io["xr"], io["wc1"], io["bc1v"], io["wc2"], io["bc2b"], io["wr"],
        io["brb"])
    rl_out, conf_out, sw_out, si_out = (
        io["rl"], io["conf"], io["sw"], io["si"])
    n_tt = TPC // 128
    with (
        tc.tile_pool(name="rconst", bufs=1) as cp,
        tc.tile_pool(name="rbig", bufs=1) as bp,
        tc.tile_pool(name="rwork", bufs=2) as rp,
        tc.tile_pool(name="rpsum", bufs=2, space="PSUM") as pr,
    ):
        xr_sb = cp.tile([128, KT_H, TPC], F32, tag="xr")
        wc1_sb = cp.tile([128, KT_H, HC], F32, tag="wc1")
        wr_sb = cp.tile([128, KT_H, E], F32, tag="wr")
        wc2_sb = cp.tile([128, KT_C, 1], F32, tag="wc2")
        bc1_sb = cp.tile([128, KT_C], F32, tag="bc1")
        bc2_sb = cp.tile([128, 1], F32, tag="bc2")
        brb_sb = cp.tile([128, E], F32, tag="brb")
        for k in range(KT_H):
            nc.sync.dma_start(xr_sb[:, k], xr[k * 128:(k + 1) * 128, :])
            nc.sync.dma_start(wc1_sb[:, k], wc1[k * 128:(k + 1) * 128, :])
            nc.sync.dma_start(wr_sb[:, k], wr[k * 128:(k + 1) * 128, :])
        for k in range(KT_C):
            nc.sync.dma_start(wc2_sb[:, k], wc2[k * 128:(k + 1) * 128, :])
        nc.sync.dma_start(bc1_sb[:], bc1v.rearrange("(m p) -> p m", p=128))
        nc.sync.dma_start(bc2_sb[:], bc2b[:, :])
        nc.sync.dma_start(brb_sb[:], brb[:, :])

        # conf hidden: c1 = relu(Wc1.T @ x + bc1)   [HC, TPC] feature-major
        c1_sb = bp.tile([128, KT_C, TPC], F32, tag="c1")
        for m in range(KT_C):
            for s in range(TPC // 512):
                ps = pr.tile([128, 512], F32, tag="ps_c1")
                for k in range(KT_H):
                    nc.tensor.matmul(
                        ps[:], wc1_sb[:, k, m * 128:(m + 1) * 128],
                        xr_sb[:, k, s * 512:(s + 1) * 512],
                        start=(k == 0), stop=(k == KT_H - 1))
                nc.scalar.activation(
                    c1_sb[:, m, s * 512:(s + 1) * 512], ps[:], AF.Relu,
                    bias=bc1_sb[:, m:m + 1])

        # router logits for all token tiles -> logits3 [128, n_tt, E]
        logits3 = bp.tile([128, n_tt, E], F32, tag="logits3")
        for t in range(n_tt):
            tsl = slice(t * 128, (t + 1) * 128)
            ps_l = pr.tile([128, E], F32, tag="ps_l")
            for k in range(KT_H):
                nc.tensor.matmul(ps_l[:], xr_sb[:, k, tsl], wr_sb[:, k],
                                 start=(k == 0), stop=(k == KT_H - 1))
            nc.vector.tensor_tensor(logits3[:, t], ps_l[:], brb_sb[:],
                                    ALU.add)
        nc.sync.dma_start(
            rl_out.rearrange("(t p) e -> p t e", p=128), logits3[:])

        # conf logits for all tiles -> conf_all [128, n_tt] (token-major)
        conf_all = bp.tile([128, n_tt], F32, tag="conf_all")
        for t in range(n_tt):
            tsl = slice(t * 128, (t + 1) * 128)
            ps_c = pr.tile([128, 1], F32, tag="ps_c")
            for k in range(KT_C):
                nc.tensor.matmul(ps_c[:], c1_sb[:, k, tsl], wc2_sb[:, k],
                                 start=(k == 0), stop=(k == KT_C - 1))
            nc.scalar.activation(conf_all[:, t:t + 1], ps_c[:], AF.Sigmoid,
                                 bias=bc2_sb[:])
        nc.sync.dma_start(
            conf_out.rearrange("(t p) o -> p t o", p=128),
            conf_all[:, :, None])

        # batched softmax over E for all tiles
        mx = rp.tile([128, n_tt], F32, tag="mx")
        nc.vector.reduce_max(mx[:], logits3[:], axis=AX.X)
        cen = rp.tile([128, n_tt, E], F32, tag="cen")
        nc.vector.tensor_tensor(
            cen[:], logits3[:], mx[:, :, None].to_broadcast([128, n_tt, E]),
            ALU.subtract)
        ex = rp.tile([128, n_tt, E], F32, tag="ex")
        nc.scalar.activation(ex[:], cen[:], AF.Exp)
        sm = rp.tile([128, n_tt], F32, tag="sm")
        nc.vector.reduce_sum(sm[:], ex[:], axis=AX.X)
        rs = rp.tile([128, n_tt], F32, tag="rs")
        nc.vector.reciprocal(rs[:], sm[:])
        probs3 = rp.tile([128, n_tt, E], F32, tag="probs3")
        nc.vector.tensor_tensor(
            probs3[:], ex[:], rs[:, :, None].to_broadcast([128, n_tt, E]),
            ALU.mult)

        # full sort of the 8 probs per token (desc) + indices
        sv3 = rp.tile([128, n_tt, 8], F32, tag="sv3")
        si3 = rp.tile([128, n_tt, 8], U32, tag="si3")
        for t in range(n_tt):
            nc.vector.max(sv3[:, t], probs3[:, t])
            nc.vector.max_index(si3[:, t], sv3[:, t], probs3[:, t])

        # dyn_k validity: slot k valid iff round(4-3c) > k iff 4-3c >= k+0.5
        v_all = rp.tile([128, n_tt], F32, tag="v_all")
        nc.scalar.activation(v_all[:], conf_all[:], AF.Copy, bias=4.0,
                             scale=-3.0)
        sw3 = rp.tile([128, n_tt, MAX_K], F32, tag="sw3")
        nc.vector.tensor_copy(sw3[:], sv3[:, :, :MAX_K])
        si_f3 = rp.tile([128, n_tt, MAX_K], F32, tag="si_f3")
        nc.vector.tensor_copy(si_f3[:], si3[:, :, :MAX_K])
        for k in range(1, MAX_K):
            mk = rp.tile([128, n_tt], F32, tag="mk")
            nc.vector.tensor_scalar(mk[:], v_all[:], float(k) + 0.5,
                                    scalar2=None, op0=ALU.is_ge)
            nc.vector.tensor_tensor(sw3[:, :, k], sw3[:, :, k], mk[:],
                                    ALU.mult)
            nc.vector.tensor_tensor(si_f3[:, :, k], si_f3[:, :, k], mk[:],
                                    ALU.mult)
        si_i3 = rp.tile([128, n_tt, MAX_K], I32, tag="si_i3")
        nc.vector.tensor_copy(si_i3[:], si_f3[:])
        nc.sync.dma_start(
            sw_out.rearrange("(t p) s -> p t s", p=128), sw3[:])
        nc.sync.dma_start(
            si_out.rearrange("(t p) s -> p t s", p=128), si_i3[:])


def _emit_ffn(nc, tc, io, cap):
    xg, w1a, w2a, y_out = io["xg"], io["w1a"], io["w2a"], io["y"]
    # chunks of 512 plus an optional 256 tail (f32r keeps full rate at >=256)
    chunks = [CHUNK] * (cap // CHUNK)
    if cap % CHUNK:
        chunks.append(cap % CHUNK)
    with (
        tc.tile_pool(name="fx", bufs=2) as fx,
        tc.tile_pool(name="fh", bufs=1) as fh,
        tc.tile_pool(name="fw1", bufs=3) as fw1,
        tc.tile_pool(name="fw2", bufs=2) as fw2,
        tc.tile_pool(name="fy", bufs=3) as fy,
        tc.tile_pool(name="fpsum", bufs=4, space="PSUM") as fp,
    ):
        c0 = 0
        for cw in chunks:
            csl = slice(c0, c0 + cw)
            c0 += cw
            xg_full = fx.tile([128, KT_HA, CHUNK], F32R, tag="xg")
            xg_sb = xg_full[:, :, :cw]
            for k in range(KT_HA):
                nc.sync.dma_start(xg_sb[:, k], xg[k * 128:(k + 1) * 128, csl])
            h_full = fh.tile([128, KT_I, CHUNK], F32R, tag="h")
            h_sb = h_full[:, :, :cw]
            for i in range(KT_I):
                w1t = fw1.tile([128, KT_HA, 128], F32R, tag="w1t")
                nc.sync.dma_start(
                    w1t[:],
                    w1a[:, i * 128:(i + 1) * 128].rearrange(
                        "(k p) m -> p k m", p=128))
                ps_full = fp.tile([128, CHUNK], F32, tag="psA")
                ps = ps_full[:, :cw]
                for k in range(KT_HA):
                    nc.tensor.matmul(ps[:], w1t[:, k], xg_sb[:, k],
                                     start=(k == 0), stop=(k == KT_HA - 1))
                nc.scalar.activation(h_sb[:, i], ps[:], AF.Relu)
            for hd in range(H // 128):
                w2t = fw2.tile([128, KT_IA, 128], F32R, tag="w2t")
                nc.sync.dma_start(
                    w2t[:, :KT_I],
                    w2a[:I, hd * 128:(hd + 1) * 128].rearrange(
                        "(k p) m -> p k m", p=128))
                nc.sync.dma_start(w2t[:, KT_I],
                                  w2a[I:IA, hd * 128:(hd + 1) * 128])
                ps2_full = fp.tile([128, CHUNK], F32, tag="psB")
                ps2 = ps2_full[:, :cw]
                for k2 in range(KT_I):
                    nc.tensor.matmul(ps2[:], w2t[:, k2], h_sb[:, k2],
                                     start=(k2 == 0), stop=False)
                # bias row: rhs = [g; 0...] block of xg, lhsT = [b2; 0...]
                nc.tensor.matmul(ps2[:], w2t[:, KT_I], xg_sb[:, KT_H],
                                 start=False, stop=True)
                y_full = fy.tile([128, CHUNK], F32, tag="y")
                y_sb = y_full[:, :cw]
                nc.vector.tensor_copy(y_sb[:], ps2[:])
                nc.sync.dma_start(y_out[hd * 128:(hd + 1) * 128, csl],
                                  y_sb[:])


def _build_program(cap: int, reps: int = 1, do_routing: bool = True,
                   do_ffn: bool = True):
    """One SPMD program, identical on all 8 cores (core e owns expert e).
    reps>1 replicates the whole compute body for differential timing."""
    nc = bacc.Bacc("TRN2", target_bir_lowering=False, debug=False)

    io = {
        # FFN inputs (per-core = per-expert)
        "xg": nc.dram_tensor("xg", [HA, cap], F32R, kind="ExternalInput").ap(),
        "w1a": nc.dram_tensor("w1a", [HA, I], F32R, kind="ExternalInput").ap(),
        "w2a": nc.dram_tensor("w2a", [IA, H], F32R, kind="ExternalInput").ap(),
        # routing inputs (x slice per-core; small nets replicated)
        "xr": nc.dram_tensor("xr", [H, TPC], F32, kind="ExternalInput").ap(),
        "wc1": nc.dram_tensor("wc1", [H, HC], F32, kind="ExternalInput").ap(),
        "bc1v": nc.dram_tensor("bc1v", [HC], F32, kind="ExternalInput").ap(),
        "wc2": nc.dram_tensor("wc2", [HC, 1], F32, kind="ExternalInput").ap(),
        "bc2b": nc.dram_tensor("bc2b", [128, 1], F32,
                               kind="ExternalInput").ap(),
        "wr": nc.dram_tensor("wr", [H, E], F32, kind="ExternalInput").ap(),
        "brb": nc.dram_tensor("brb", [128, E], F32, kind="ExternalInput").ap(),
        # outputs
        "y": nc.dram_tensor("y", [H, cap], F32, kind="ExternalOutput").ap(),
        "rl": nc.dram_tensor("rl", [TPC, E], F32, kind="ExternalOutput").ap(),
        "conf": nc.dram_tensor("conf", [TPC, 1], F32,
                               kind="ExternalOutput").ap(),
        "sw": nc.dram_tensor("sw", [TPC, MAX_K], F32,
                             kind="ExternalOutput").ap(),
        "si": nc.dram_tensor("si", [TPC, MAX_K], I32,
                             kind="ExternalOutput").ap(),
    }

    with tile.TileContext(nc) as tc:
        for _rep in range(reps):
            if do_routing:
                _emit_routing(nc, tc, io)
            if do_ffn:
                _emit_ffn(nc, tc, io, cap)
    nc.compile()
    return nc


def _host_routing(x, Wr, br, Wc1, bc1, Wc2, bc2):
    """fp32 routing on host — used only to DISPATCH tokens to experts."""
    logits = (x @ Wr + br) / TEMPERATURE
    m = logits.max(axis=-1, keepdims=True)
    e = np.exp(logits - m)
    probs = e / e.sum(axis=-1, keepdims=True)
    order = np.argsort(-probs, axis=-1, kind="stable")
    top2 = order[:, :TOP_K]
    bw = np.take_along_axis(probs, top2, axis=-1)
    bwn = bw / bw.sum(axis=-1, keepdims=True)
    return top2.astype(np.int64), bwn.astype(np.float32)


def prepare(hidden_states, Wr, br, Wc1, bc1, Wc2, bc2, W1, b1, W2, b2,
            reps: int = 1, do_routing: bool = True, do_ffn: bool = True):
    """Host sharding: routing + dispatch + per-core input maps.
    Returns (nc, in_maps, meta) where meta is needed by combine()."""
    hidden_states = np.asarray(hidden_states, dtype=np.float32)
    Wr = np.asarray(Wr, np.float32); br = np.asarray(br, np.float32)
    Wc1 = np.asarray(Wc1, np.float32); bc1 = np.asarray(bc1, np.float32)
    Wc2 = np.asarray(Wc2, np.float32); bc2 = np.asarray(bc2, np.float32)
    W1 = np.asarray(W1, np.float32); b1 = np.asarray(b1, np.float32)
    W2 = np.asarray(W2, np.float32); b2 = np.asarray(b2, np.float32)

    x = hidden_states.reshape(-1, H)                       # [N_TOK, H]
    top2, bwn = _host_routing(x, Wr, br, Wc1, bc1, Wc2, bc2)

    # --- dispatch: token/gate lists per expert ---
    toks_e, gates_e = [], []
    for e in range(E):
        m = top2 == e                                      # [N, 2]
        sel = np.nonzero(m.any(axis=1))[0]
        slot = np.argmax(m[sel], axis=1)
        toks_e.append(sel)
        gates_e.append(bwn[sel, slot])
    max_cnt = max(len(t) for t in toks_e)
    cap = max(256, int(math.ceil(max_cnt / 256)) * 256)

    key = (cap, reps, do_routing, do_ffn)
    if key not in _prog_cache:
        _prog_cache[key] = _build_program(cap, reps, do_routing, do_ffn)
    nc = _prog_cache[key]

    # --- per-core input maps ---
    xT = np.ascontiguousarray(x.T)                         # [H, N_TOK]
    bc2b = np.broadcast_to(bc2.reshape(1, 1), (128, 1)).copy()
    brb = np.broadcast_to(br.reshape(1, E), (128, E)).copy()
    in_maps = []
    for e in range(E):
        toks, g = toks_e[e], gates_e[e]
        cnt = len(toks)
        xg = np.zeros((HA, cap), np.float32)
        xg[:H, :cnt] = xT[:, toks] * g[None, :]
        xg[H, :cnt] = g
        w1a = np.zeros((HA, I), np.float32)
        w1a[:H] = W1[e]
        w1a[H] = b1[e]
        w2a = np.zeros((IA, H), np.float32)
        w2a[:I] = W2[e]
        w2a[I] = b2[e]
        in_maps.append({
            "xg": xg, "w1a": w1a, "w2a": w2a,
            "xr": np.ascontiguousarray(xT[:, e * TPC:(e + 1) * TPC]),
            "wc1": Wc1, "bc1v": bc1, "wc2": Wc2.reshape(HC, 1),
            "bc2b": bc2b, "wr": Wr, "brb": brb,
        })
    return nc, in_maps, (toks_e, top2, cap)


def combine(results, meta):
    """Unshard the per-core results into the full reference-shaped outputs."""
    toks_e, top2, cap = meta
    rl = np.concatenate([r["rl"] for r in results], axis=0)          # [N, E]
    conf = np.concatenate([r["conf"][:, 0] for r in results], axis=0)
    sw = np.concatenate([r["sw"] for r in results], axis=0)
    si = np.concatenate([r["si"] for r in results], axis=0)

    y_cat = np.concatenate([r["y"] for r in results], axis=1)        # [H, E*cap]
    flat = np.zeros((N_TOK, TOP_K), np.int64)
    for e in range(E):
        toks = toks_e[e]
        slot = np.argmax(top2[toks] == e, axis=1)
        flat[toks, slot] = e * cap + np.arange(len(toks))
    out = (y_cat[:, flat[:, 0]] + y_cat[:, flat[:, 1]]).T            # [N, H]

    return (
        out.reshape(B, S, H),
        sw.reshape(B, S, MAX_K),
        si.reshape(B, S, MAX_K).astype(np.int32),
        conf,
        rl,
    )


def kernel(**inputs):
    nc, in_maps, meta = prepare(**inputs)
    results = run_bass_kernel_spmd(nc, in_maps, list(range(N_CORES))).results
    return combine(results, meta)


# revision 20
# speedup vs baseline: 3.4203x; 3.4203x over previous
"""MoE++ layer (nn_MoEPlusPlusLayer) on 8 Trainium2 NeuronCores.

Strategy (expert-parallel, per the sharding hint):
  - Host computes the fp32 routing math once to DISPATCH tokens by expert id
    (the sharding step): each of the 8 cores owns one expert's FFN weights and
    receives that expert's assigned tokens as a capacity-padded batch.
  - Gates/biases are folded exactly into the token batch: xg = [g * x^T; g; 0]
    and W1a = [W1; b1; 0], W2a = [W2; b2; 0], using relu(g*z) = g*relu(z) for
    g >= 0. The device FFN is then pure matmul -> relu -> matmul (float32r on
    the PE array at full rate, ~1e-4 relative error).
  - Each core also computes the routing OUTPUTS (router_logits, conf,
    selected_weights, selected_indices) for its 1/8 slice of tokens in fp32
    (data-parallel over the batch, small nets replicated).
  - Host unshards: concatenates routing outputs and combines the two gated
    expert contributions per token with two gathers + add.
"""

import math

import numpy as np

import concourse.bass as bass
import concourse.mybir as mybir
import concourse.tile as tile
from concourse import bacc
from concourse.bass_utils import run_bass_kernel_spmd

# Model dims (fixed by the problem)
B, S, H = 4, 2048, 1024
E, I = 8, 4096
TOP_K = 2
MIN_K, MAX_K = 1, 4
TEMPERATURE = 1.0

N_CORES = 8
N_TOK = B * S                  # 8192
TPC = N_TOK // N_CORES         # tokens per core for the routing math
HC = H // 2                    # confidence hidden dim (512)
KT_H = H // 128                # 8 k-tiles over H
KT_HA = KT_H + 1               # 9 k-tiles over augmented H (bias row)
KT_I = I // 128                # 32 k-tiles over I
KT_IA = KT_I + 1               # 33 k-tiles over augmented I (bias row)
KT_C = HC // 128               # 4 k-tiles over confidence hidden
HA = KT_HA * 128               # 1152 augmented input rows
IA = KT_IA * 128               # 4224 augmented intermediate rows
CHUNK = 1024                   # token chunk held in SBUF (2x 512 mm slices)

F32 = mybir.dt.float32
F32R = mybir.dt.float32r
U32 = mybir.dt.uint32
I32 = mybir.dt.int32
FDT = mybir.dt.float16         # FFN compute dtype (fp16: full PE rate,
NPDT = np.float16              # ~3e-4 rel; gates/biases folded exactly)
AF = mybir.ActivationFunctionType
AX = mybir.AxisListType
ALU = mybir.AluOpType

_prog_cache: dict = {}


def _emit_routing(nc, tc, io):
    """Routing outputs for this core's TPC tokens. All per-token-tile math is
    batched into 3D [128, n_tt, E] tiles to minimize instruction count and
    ACT function-table switches."""
    xr, wc1, bc1v, wc2, bc2b, wr, brb = (
        io["xr"], io["wc1"], io["bc1v"], io["wc2"], io["bc2b"], io["wr"],
        io["brb"])
    rl_out, conf_out, sw_out, si_out = (
        io["rl"], io["conf"], io["sw"], io["si"])
    n_tt = TPC // 128
    with (
        tc.tile_pool(name="rconst", bufs=1) as cp,
        tc.tile_pool(name="rbig", bufs=1) as bp,
        tc.tile_pool(name="rwork", bufs=2) as rp,
        tc.tile_pool(name="rpsum", bufs=2, space="PSUM") as pr,
    ):
        xr_sb = cp.tile([128, KT_H, TPC], F32, tag="xr")
        wc1_sb = cp.tile([128, KT_H, HC], F32, tag="wc1")
        wr_sb = cp.tile([128, KT_H, E], F32, tag="wr")
        wc2_sb = cp.tile([128, KT_C, 1], F32, tag="wc2")
        bc1_sb = cp.tile([128, KT_C], F32, tag="bc1")
        bc2_sb = cp.tile([128, 1], F32, tag="bc2")
        brb_sb = cp.tile([128, E], F32, tag="brb")
        for k in range(KT_H):
            nc.sync.dma_start(xr_sb[:, k], xr[k * 128:(k + 1) * 128, :])
            nc.sync.dma_start(wc1_sb[:, k], wc1[k * 128:(k + 1) * 128, :])
            nc.sync.dma_start(wr_sb[:, k], wr[k * 128:(k + 1) * 128, :])
        for k in range(KT_C):
            nc.sync.dma_start(wc2_sb[:, k], wc2[k * 128:(k + 1) * 128, :])
        nc.sync.dma_start(bc1_sb[:], bc1v.rearrange("(m p) -> p m", p=128))
        nc.sync.dma_start(bc2_sb[:], bc2b[:, :])
        nc.sync.dma_start(brb_sb[:], brb[:, :])

        # conf hidden: c1 = relu(Wc1.T @ x + bc1)   [HC, TPC] feature-major
        c1_sb = bp.tile([128, KT_C, TPC], F32, tag="c1")
        for m in range(KT_C):
            for s in range(TPC // 512):
                ps = pr.tile([128, 512], F32, tag="ps_c1")
                for k in range(KT_H):
                    nc.tensor.matmul(
                        ps[:], wc1_sb[:, k, m * 128:(m + 1) * 128],
                        xr_sb[:, k, s * 512:(s + 1) * 512],
                        start=(k == 0), stop=(k == KT_H - 1))
                nc.scalar.activation(
                    c1_sb[:, m, s * 512:(s + 1) * 512], ps[:], AF.Relu,
                    bias=bc1_sb[:, m:m + 1])

        # router logits for all token tiles -> logits3 [128, n_tt, E]
        logits3 = bp.tile([128, n_tt, E], F32, tag="logits3")
        for t in range(n_tt):
            tsl = slice(t * 128, (t + 1) * 128)
            ps_l = pr.tile([128, E], F32, tag="ps_l")
            for k in range(KT_H):
                nc.tensor.matmul(ps_l[:], xr_sb[:, k, tsl], wr_sb[:, k],
                                 start=(k == 0), stop=(k == KT_H - 1))
            nc.vector.tensor_tensor(logits3[:, t], ps_l[:], brb_sb[:],
                                    ALU.add)
        nc.sync.dma_start(
            rl_out.rearrange("(t p) e -> p t e", p=128), logits3[:])

        # conf logits for all tiles -> conf_all [128, n_tt] (token-major)
        conf_all = bp.tile([128, n_tt], F32, tag="conf_all")
        for t in range(n_tt):
            tsl = slice(t * 128, (t + 1) * 128)
            ps_c = pr.tile([128, 1], F32, tag="ps_c")
            for k in range(KT_C):
                nc.tensor.matmul(ps_c[:], c1_sb[:, k, tsl], wc2_sb[:, k],
                                 start=(k == 0), stop=(k == KT_C - 1))
            nc.scalar.activation(conf_all[:, t:t + 1], ps_c[:], AF.Sigmoid,
                                 bias=bc2_sb[:])
        nc.sync.dma_start(
            conf_out.rearrange("(t p) o -> p t o", p=128),
            conf_all[:, :, None])

        # batched softmax over E for all tiles
        mx = rp.tile([128, n_tt], F32, tag="mx")
        nc.vector.reduce_max(mx[:], logits3[:], axis=AX.X)
        cen = rp.tile([128, n_tt, E], F32, tag="cen")
        nc.vector.tensor_tensor(
            cen[:], logits3[:], mx[:, :, None].to_broadcast([128, n_tt, E]),
            ALU.subtract)
        ex = rp.tile([128, n_tt, E], F32, tag="ex")
        nc.scalar.activation(ex[:], cen[:], AF.Exp)
        sm = rp.tile([128, n_tt], F32, tag="sm")
        nc.vector.reduce_sum(sm[:], ex[:], axis=AX.X)
        rs = rp.tile([128, n_tt], F32, tag="rs")
        nc.vector.reciprocal(rs[:], sm[:])
        probs3 = rp.tile([128, n_tt, E], F32, tag="probs3")
        nc.vector.tensor_tensor(
            probs3[:], ex[:], rs[:, :, None].to_broadcast([128, n_tt, E]),
            ALU.mult)

        # full sort of the 8 probs per token (desc) + indices
        sv3 = rp.tile([128, n_tt, 8], F32, tag="sv3")
        si3 = rp.tile([128, n_tt, 8], U32, tag="si3")
        for t in range(n_tt):
            nc.vector.max(sv3[:, t], probs3[:, t])
            nc.vector.max_index(si3[:, t], sv3[:, t], probs3[:, t])

        # dyn_k validity: slot k valid iff round(4-3c) > k iff 4-3c >= k+0.5
        v_all = rp.tile([128, n_tt], F32, tag="v_all")
        nc.scalar.activation(v_all[:], conf_all[:], AF.Copy, bias=4.0,
                             scale=-3.0)
        sw3 = rp.tile([128, n_tt, MAX_K], F32, tag="sw3")
        nc.vector.tensor_copy(sw3[:], sv3[:, :, :MAX_K])
        si_f3 = rp.tile([128, n_tt, MAX_K], F32, tag="si_f3")
        nc.vector.tensor_copy(si_f3[:], si3[:, :, :MAX_K])
        for k in range(1, MAX_K):
            mk = rp.tile([128, n_tt], F32, tag="mk")
            nc.vector.tensor_scalar(mk[:], v_all[:], float(k) + 0.5,
                                    scalar2=None, op0=ALU.is_ge)
            nc.vector.tensor_tensor(sw3[:, :, k], sw3[:, :, k], mk[:],
                                    ALU.mult)
            nc.vector.tensor_tensor(si_f3[:, :, k], si_f3[:, :, k], mk[:],
                                    ALU.mult)
        si_i3 = rp.tile([128, n_tt, MAX_K], I32, tag="si_i3")
        nc.vector.tensor_copy(si_i3[:], si_f3[:])
        nc.sync.dma_start(
            sw_out.rearrange("(t p) s -> p t s", p=128), sw3[:])
        nc.sync.dma_start(
            si_out.rearrange("(t p) s -> p t s", p=128), si_i3[:])


def _slices(w):
    out, o = [], 0
    while o < w:
        s = min(512, w - o)
        out.append(slice(o, o + s))
        o += s
    return out


def _emit_ffn(nc, tc, io, cap):
    """FFN in fp16 (inputs/weights host-pre-tiled for contiguous DMA):
    xgd [128, KT_HA, cap], w1d [8, 128, KT_HA, 512], w2d [8, 128, KT_IA, 128],
    y_out [128, 8, cap] (all feature tiles on partitions)."""
    xgd, w1d, w2d, y_out = io["xgd"], io["w1d"], io["w2d"], io["y"]
    chunks = [CHUNK] * (cap // CHUNK)
    if cap % CHUNK:
        chunks.append(cap % CHUNK)
    with (
        tc.tile_pool(name="fx", bufs=2) as fx,
        tc.tile_pool(name="fh", bufs=1) as fh,
        tc.tile_pool(name="fw1", bufs=3) as fw1,
        tc.tile_pool(name="fw2", bufs=3) as fw2,
        tc.tile_pool(name="fy", bufs=3) as fy,
        tc.tile_pool(name="fpsum", bufs=4, space="PSUM") as fp,
    ):
        c0 = 0
        for cw in chunks:
            csl = slice(c0, c0 + cw)
            c0 += cw
            xg_full = fx.tile([128, KT_HA, CHUNK], FDT, tag="xg")
            xg_sb = xg_full[:, :, :cw]
            nc.sync.dma_start(xg_sb[:], xgd[:, :, csl])
            h_full = fh.tile([128, KT_I, CHUNK], FDT, tag="h")
            h_sb = h_full[:, :, :cw]
            for ib in range(KT_I // 4):
                w1t = fw1.tile([128, KT_HA, 512], FDT, tag="w1t")
                nc.sync.dma_start(w1t[:], w1d[ib])
                for j in range(4):
                    i = ib * 4 + j
                    jsl = slice(j * 128, (j + 1) * 128)
                    for ssl in _slices(cw):
                        ps_full = fp.tile([128, 512], F32, tag="psA")
                        ps = ps_full[:, :ssl.stop - ssl.start]
                        for k in range(KT_HA):
                            nc.tensor.matmul(
                                ps[:], w1t[:, k, jsl], xg_sb[:, k, ssl],
                                start=(k == 0), stop=(k == KT_HA - 1))
                        nc.scalar.activation(h_sb[:, i, ssl], ps[:], AF.Relu)
            for hd in range(H // 128):
                w2t = fw2.tile([128, KT_IA, 128], FDT, tag="w2t")
                nc.sync.dma_start(w2t[:], w2d[hd])
                y_full = fy.tile([128, CHUNK], F32, tag="y")
                y_sb = y_full[:, :cw]
                for ssl in _slices(cw):
                    ps2_full = fp.tile([128, 512], F32, tag="psB")
                    ps2 = ps2_full[:, :ssl.stop - ssl.start]
                    for k2 in range(KT_I):
                        nc.tensor.matmul(ps2[:], w2t[:, k2], h_sb[:, k2, ssl],
                                         start=(k2 == 0), stop=False)
                    # bias row: rhs = [g; 0...] block of xg, lhsT = [b2; 0...]
                    nc.tensor.matmul(ps2[:], w2t[:, KT_I],
                                     xg_sb[:, KT_H, ssl],
                                     start=False, stop=True)
                    nc.vector.tensor_copy(y_sb[:, ssl], ps2[:])
                nc.sync.dma_start(y_out[:, hd, csl], y_sb[:])


def _build_program(cap: int, reps: int = 1, do_routing: bool = True,
                   do_ffn: bool = True):
    """One SPMD program, identical on all 8 cores (core e owns expert e).
    reps>1 replicates the whole compute body for differential timing."""
    nc = bacc.Bacc("TRN2", target_bir_lowering=False, debug=False)

    io = {
        # FFN inputs (per-core = per-expert), host-pre-tiled for contig DMA
        "xgd": nc.dram_tensor("xgd", [128, KT_HA, cap], FDT,
                              kind="ExternalInput").ap(),
        "w1d": nc.dram_tensor("w1d", [KT_I // 4, 128, KT_HA, 512], FDT,
                              kind="ExternalInput").ap(),
        "w2d": nc.dram_tensor("w2d", [H // 128, 128, KT_IA, 128], FDT,
                              kind="ExternalInput").ap(),
        # routing inputs (x slice per-core; small nets replicated)
        "xr": nc.dram_tensor("xr", [H, TPC], F32, kind="ExternalInput").ap(),
        "wc1": nc.dram_tensor("wc1", [H, HC], F32, kind="ExternalInput").ap(),
        "bc1v": nc.dram_tensor("bc1v", [HC], F32, kind="ExternalInput").ap(),
        "wc2": nc.dram_tensor("wc2", [HC, 1], F32, kind="ExternalInput").ap(),
        "bc2b": nc.dram_tensor("bc2b", [128, 1], F32,
                               kind="ExternalInput").ap(),
        "wr": nc.dram_tensor("wr", [H, E], F32, kind="ExternalInput").ap(),
        "brb": nc.dram_tensor("brb", [128, E], F32, kind="ExternalInput").ap(),
        # outputs
        "y": nc.dram_tensor("y", [128, H // 128, cap], F32,
                            kind="ExternalOutput").ap(),
        "rl": nc.dram_tensor("rl", [TPC, E], F32, kind="ExternalOutput").ap(),
        "conf": nc.dram_tensor("conf", [TPC, 1], F32,
                               kind="ExternalOutput").ap(),
        "sw": nc.dram_tensor("sw", [TPC, MAX_K], F32,
                             kind="ExternalOutput").ap(),
        "si": nc.dram_tensor("si", [TPC, MAX_K], I32,
                             kind="ExternalOutput").ap(),
    }

    with tile.TileContext(nc) as tc:
        def body():
            if do_routing:
                _emit_routing(nc, tc, io)
            if do_ffn:
                _emit_ffn(nc, tc, io, cap)
        if reps == 1:
            body()
        else:
            with tc.For_i(0, reps, 1):
                body()
    nc.compile()
    return nc


def _host_routing(x, Wr, br, Wc1, bc1, Wc2, bc2):
    """fp32 routing on host — used only to DISPATCH tokens to experts."""
    logits = (x @ Wr + br) / TEMPERATURE
    m = logits.max(axis=-1, keepdims=True)
    e = np.exp(logits - m)
    probs = e / e.sum(axis=-1, keepdims=True)
    order = np.argsort(-probs, axis=-1, kind="stable")
    top2 = order[:, :TOP_K]
    bw = np.take_along_axis(probs, top2, axis=-1)
    bwn = bw / bw.sum(axis=-1, keepdims=True)
    return top2.astype(np.int64), bwn.astype(np.float32)


def prepare(hidden_states, Wr, br, Wc1, bc1, Wc2, bc2, W1, b1, W2, b2,
            reps: int = 1, do_routing: bool = True, do_ffn: bool = True):
    """Host sharding: routing + dispatch + per-core input maps.
    Returns (nc, in_maps, meta) where meta is needed by combine()."""
    hidden_states = np.asarray(hidden_states, dtype=np.float32)
    Wr = np.asarray(Wr, np.float32); br = np.asarray(br, np.float32)
    Wc1 = np.asarray(Wc1, np.float32); bc1 = np.asarray(bc1, np.float32)
    Wc2 = np.asarray(Wc2, np.float32); bc2 = np.asarray(bc2, np.float32)
    W1 = np.asarray(W1, np.float32); b1 = np.asarray(b1, np.float32)
    W2 = np.asarray(W2, np.float32); b2 = np.asarray(b2, np.float32)

    x = hidden_states.reshape(-1, H)                       # [N_TOK, H]
    top2, bwn = _host_routing(x, Wr, br, Wc1, bc1, Wc2, bc2)

    # --- dispatch: token/gate lists per expert ---
    toks_e, gates_e = [], []
    for e in range(E):
        m = top2 == e                                      # [N, 2]
        sel = np.nonzero(m.any(axis=1))[0]
        slot = np.argmax(m[sel], axis=1)
        toks_e.append(sel)
        gates_e.append(bwn[sel, slot])
    max_cnt = max(len(t) for t in toks_e)
    cap = max(256, int(math.ceil(max_cnt / 256)) * 256)

    key = (cap, reps, do_routing, do_ffn)
    if key not in _prog_cache:
        _prog_cache[key] = _build_program(cap, reps, do_routing, do_ffn)
    nc = _prog_cache[key]

    # --- per-core input maps ---
    xT = np.ascontiguousarray(x.T)                         # [H, N_TOK]
    bc2b = np.broadcast_to(bc2.reshape(1, 1), (128, 1)).copy()
    brb = np.broadcast_to(br.reshape(1, E), (128, E)).copy()
    in_maps = []
    for e in range(E):
        toks, g = toks_e[e], gates_e[e]
        cnt = len(toks)
        xg = np.zeros((HA, cap), NPDT)
        xg[:H, :cnt] = (xT[:, toks] * g[None, :]).astype(NPDT)
        xg[H, :cnt] = g.astype(NPDT)
        # tile: xgd[p, k, m] = xg[k*128+p, m]
        xgd = np.ascontiguousarray(
            xg.reshape(KT_HA, 128, cap).transpose(1, 0, 2))
        w1a = np.zeros((HA, I), NPDT)
        w1a[:H] = W1[e].astype(NPDT)
        w1a[H] = b1[e].astype(NPDT)
        # tile: w1d[ib, p, k, m] = w1a[k*128+p, ib*512+m]
        w1d = np.ascontiguousarray(
            w1a.reshape(KT_HA, 128, KT_I // 4, 512).transpose(2, 1, 0, 3))
        w2a = np.zeros((IA, H), NPDT)
        w2a[:I] = W2[e].astype(NPDT)
        w2a[I] = b2[e].astype(NPDT)
        # tile: w2d[hd, p, k, m] = w2a[k*128+p, hd*128+m]
        w2d = np.ascontiguousarray(
            w2a.reshape(KT_IA, 128, H // 128, 128).transpose(2, 1, 0, 3))
        in_maps.append({
            "xgd": xgd, "w1d": w1d, "w2d": w2d,
            "xr": np.ascontiguousarray(xT[:, e * TPC:(e + 1) * TPC]),
            "wc1": Wc1, "bc1v": bc1, "wc2": Wc2.reshape(HC, 1),
            "bc2b": bc2b, "wr": Wr, "brb": brb,
        })
    return nc, in_maps, (toks_e, top2, cap)


def combine(results, meta):
    """Unshard the per-core results into the full reference-shaped outputs."""
    toks_e, top2, cap = meta
    rl = np.concatenate([r["rl"] for r in results], axis=0)          # [N, E]
    conf = np.concatenate([r["conf"][:, 0] for r in results], axis=0)
    sw = np.concatenate([r["sw"] for r in results], axis=0)
    si = np.concatenate([r["si"] for r in results], axis=0)

    # y[p, hd, m] -> [H, cap] with H = hd*128 + p
    y_cat = np.concatenate(
        [r["y"].transpose(1, 0, 2).reshape(H, cap) for r in results],
        axis=1)                                                      # [H, E*cap]
    flat = np.zeros((N_TOK, TOP_K), np.int64)
    for e in range(E):
        toks = toks_e[e]
        slot = np.argmax(top2[toks] == e, axis=1)
        flat[toks, slot] = e * cap + np.arange(len(toks))
    out = (y_cat[:, flat[:, 0]] + y_cat[:, flat[:, 1]]).T            # [N, H]

    return (
        out.reshape(B, S, H),
        sw.reshape(B, S, MAX_K),
        si.reshape(B, S, MAX_K).astype(np.int32),
        conf,
        rl,
    )


def kernel(**inputs):
    nc, in_maps, meta = prepare(**inputs)
    results = run_bass_kernel_spmd(nc, in_maps, list(range(N_CORES))).results
    return combine(results, meta)
